# revision 1
# baseline (speedup 1.0000x reference)
"""Multi-head attention kernel for Trainium2, 8 NeuronCores.

Problem: B=4, S=2048, HID=1024, H=16 heads, D=64.
  Q = q@Wq, K = k@Wk, V = v@Wv (reshaped to heads)
  O = softmax(Q K^T / sqrt(D)) V ;  out = O @ Wo

Sharding (hardcoded): core c handles batch b=c//2 and head-half hf=c%2
(8 of 16 heads via column-parallel Wq/Wk/Wv, row-parallel Wo).  Each core
returns a partial output [S, HID]; the host sums the two head-halves per
batch.

Per-core dataflow (all matmuls on PE in float32r, TF32-like):
  phase 1 (three passes K, V, Q): PE-transpose x chunks (fp32 exact,
    identity matmul) -> xT in SBUF; project:
      K^T[e,s] = (Wk blocks)^T @ xT   (e on partitions)
      Q^T[e,s] likewise
      V[s,e]   = (xT blocks)^T @ Wv   (s on partitions, natural layout)
    V is stored bf16 interleaved with a ones column: v_sb[:, st, h, 0:64]=V,
    [...,64]=1.0 so attn@V also produces the softmax row sums.
  phase 2 per (head-pair hp, q-chunk qc): scores S^T[k,q] psum tiles via
    row-packed K=64 matmuls (two heads at partition bases 0/64); ACT exp
    (scale=1/8) drains psum -> P^T bf16 tiles; attn@V accumulates
    O^T[d,q] (+sums row 64) over 16 k-tiles; normalization:
    DVE reciprocal of sums, gpsimd partition-broadcast, DVE multiply ->
    ot_sb float32r.
  phase 3: Y[s,:] accumulated over 4 head-pair e-blocks; DVE drain; DMA out.
"""

import threading

import numpy as np

import concourse.bacc as bacc
import concourse.mybir as mybir
import concourse.tile as tile
from concourse.bass_utils import run_bass_kernel_spmd
from concourse.masks import make_identity

DT = mybir.dt
AF = mybir.ActivationFunctionType

B, S, HID, H = 4, 2048, 1024, 16
D = HID // H               # 64
E = 512                    # local hidden (8 heads)
HLOC = 8                   # heads per core
NHP = 4                    # head pairs per core
SC = 4                     # s-chunks of 512
ST = 16                    # s-tiles of 128
CB = 8                     # contraction blocks of 128 (over HID)
ET = 4                     # e-tiles of 128 in Q^T/K^T
QCW = 512                  # q-chunk width
SCALE = 1.0 / np.sqrt(np.float32(D))   # 0.125

_lock = threading.Lock()
_cache = {}


def _build(debug=False):
    nc = bacc.Bacc(None)
    xq = nc.declare_dram_parameter("xq", [S, HID], DT.float32, isOutput=False)
    xk = nc.declare_dram_parameter("xk", [S, HID], DT.float32, isOutput=False)
    xv = nc.declare_dram_parameter("xv", [S, HID], DT.float32, isOutput=False)
    wq = nc.declare_dram_parameter("wq", [HID, E], DT.float32, isOutput=False)
    wk = nc.declare_dram_parameter("wk", [HID, E], DT.float32, isOutput=False)
    wv = nc.declare_dram_parameter("wv", [HID, E], DT.float32, isOutput=False)
    wo = nc.declare_dram_parameter("wo", [E, HID], DT.float32, isOutput=False)
    y = nc.declare_dram_parameter("y", [S, HID], DT.float32, isOutput=True)
    if debug:
        dbg_qT = nc.declare_dram_parameter("dbg_qT", [128, ET, S], DT.float32, isOutput=True)
        dbg_kT = nc.declare_dram_parameter("dbg_kT", [128, ET, S], DT.float32, isOutput=True)
        dbg_v = nc.declare_dram_parameter("dbg_v", [128, ST, HLOC, D + 1], DT.float32, isOutput=True)
        dbg_ot = nc.declare_dram_parameter("dbg_ot", [128, NHP, S], DT.float32, isOutput=True)

    with tile.TileContext(nc) as tc:
        with (
            tc.tile_pool(name="const", bufs=1) as constp,
            tc.tile_pool(name="wpool", bufs=2) as wpool,
            tc.tile_pool(name="xin", bufs=5) as xinp,
            tc.tile_pool(name="xt", bufs=(1 if debug else 2)) as xtp,
            tc.tile_pool(name="qkv", bufs=1) as qkvp,
            tc.tile_pool(name="pt", bufs=1) as ptp,
            tc.tile_pool(name="norm", bufs=2) as normp,
            tc.tile_pool(name="yout", bufs=2) as youtp,
            tc.tile_pool(name="dbg", bufs=1) as dbgp,
            tc.tile_pool(name="ps1", bufs=2, space="PSUM") as ps1,
            tc.tile_pool(name="ps_s", bufs=2, space="PSUM") as ps_s,
            tc.tile_pool(name="ps_ov", bufs=2, space="PSUM") as ps_ov,
        ):
            ps_p = ps1

            # --- weights: one shared slot tag, loaded per pass ---
            def load_w(wdram):
                wsb = wpool.tile([128, CB, E], DT.float32r, tag="w")
                for cb in range(CB):
                    nc.sync.dma_start(
                        out=wsb[:, cb, :],
                        in_=wdram[cb * 128:(cb + 1) * 128, :].bitcast(DT.float32r),
                    )
                return wsb

            qT = qkvp.tile([128, ET, S], DT.bfloat16, tag="qT")
            kT = qkvp.tile([128, ET, S], DT.bfloat16, tag="kT")
            # V in natural [s, e] layout; per s-tile and head: 64 cols of V
            # plus a ones column (row sums ride the attn@V matmul).
            v_sb = qkvp.tile([128, ST, HLOC, D + 1], DT.bfloat16, tag="v")
            nc.vector.memset(v_sb[:, :, :, D:D + 1], 1.0)

            ident = constp.tile([128, 128], DT.float32)
            make_identity(nc, ident)

            def load_and_transpose(xdram, sc):
                """DMA 4 s-blocks of x, PE-transpose into xt slice [128, CB, 512]."""
                xt_sl = xtp.tile([128, CB, QCW], DT.float32r, tag="xt")
                xins = []
                for sb in range(4):
                    xi = xinp.tile([128, HID], DT.float32, tag="xin")
                    r0 = sc * QCW + sb * 128
                    nc.sync.dma_start(out=xi, in_=xdram[r0:r0 + 128, :])
                    xins.append(xi)
                for cb in range(CB):
                    tp = ps_p.tile([128, QCW], DT.float32, tag="ps1")
                    for sb in range(4):
                        nc.tensor.transpose(
                            tp[:, sb * 128:(sb + 1) * 128],
                            xins[sb][:, cb * 128:(cb + 1) * 128],
                            ident,
                        )
                    nc.vector.tensor_copy(xt_sl[:, cb, :], tp)
                return xt_sl

            # ---------------- pass K then pass V ----------------
            wk_sb = load_w(wk)
            for sc in range(SC):
                xt_sl = load_and_transpose(xk, sc)
                for et in range(ET):
                    pp = ps_p.tile([128, QCW], DT.float32, tag="ps1")
                    for cb in range(CB):
                        nc.tensor.matmul(
                            pp,
                            wk_sb[:, cb, et * 128:(et + 1) * 128],
                            xt_sl[:, cb, :],
                            start=(cb == 0),
                            stop=(cb == CB - 1),
                        )
                    nc.vector.tensor_copy(kT[:, et, sc * QCW:(sc + 1) * QCW], pp)

            wv_sb = load_w(wv)
            for sc in range(SC):
                xt_sl = load_and_transpose(xv, sc)
                for sb in range(4):
                    st = sc * 4 + sb
                    pp = ps_p.tile([128, QCW], DT.float32, tag="ps1")
                    for cb in range(CB):
                        nc.tensor.matmul(
                            pp,
                            xt_sl[:, cb, sb * 128:(sb + 1) * 128],
                            wv_sb[:, cb, :],
                            start=(cb == 0),
                            stop=(cb == CB - 1),
                        )
                    # psum [128s, 512e] -> v_sb[:, st, h, 0:64] for all 8 heads
                    nc.vector.tensor_copy(
                        v_sb[:, st, :, 0:D],
                        pp.rearrange("p (h d) -> p h d", h=HLOC),
                    )

            # ---------------- pass Q + attention rounds ----------------
            wq_sb = load_w(wq)
            # prefetch wo for phase 3 (second wpool slot; overlaps rounds)
            wo_sb = wpool.tile([128, NHP, HID], DT.float32r, tag="w")
            for eb in range(NHP):
                nc.sync.dma_start(
                    out=wo_sb[:, eb, :],
                    in_=wo[eb * 128:(eb + 1) * 128, :].bitcast(DT.float32r),
                )
            for sc in range(SC):
                xt_sl = load_and_transpose(xq, sc)
                for et in range(ET):
                    pp = ps_p.tile([128, QCW], DT.float32, tag="ps1")
                    for cb in range(CB):
                        nc.tensor.matmul(
                            pp,
                            wq_sb[:, cb, et * 128:(et + 1) * 128],
                            xt_sl[:, cb, :],
                            start=(cb == 0),
                            stop=(cb == CB - 1),
                        )
                    nc.vector.tensor_copy(qT[:, et, sc * QCW:(sc + 1) * QCW], pp)

            if debug:
                for et in range(ET):
                    for c4 in range(4):
                        csl = slice(c4 * 512, (c4 + 1) * 512)
                        dq = dbgp.tile([128, 512], DT.float32, tag="dbg")
                        nc.vector.tensor_copy(dq, qT[:, et, csl])
                        nc.sync.dma_start(out=dbg_qT[:, et, csl], in_=dq)
                        dk = dbgp.tile([128, 512], DT.float32, tag="dbg")
                        nc.vector.tensor_copy(dk, kT[:, et, csl])
                        nc.sync.dma_start(out=dbg_kT[:, et, csl], in_=dk)
                for st in range(ST):
                    dv = dbgp.tile([128, HLOC, D + 1], DT.float32, tag="dbg2")
                    nc.vector.tensor_copy(dv, v_sb[:, st, :, :])
                    nc.sync.dma_start(out=dbg_v[:, st, :, :], in_=dv)

            ot_sb = qkvp.tile([128, NHP, S], DT.float32r, tag="ot")
            qsl = slice(0, QCW)

            for qc in range(SC):
                q0 = qc * QCW
                for hp in range(NHP):
                    ovs = [
                        ps_ov.tile([D + 1, QCW], DT.float32, tag="ps_ov",
                                   name=f"ov{qc}_{hp}_{i}")
                        for i in range(2)
                    ]
                    # k-halves keep the live P^T set at 8 k-tiles x 2 heads.
                    # Both heads' score tiles share one 2-bank psum tile so a
                    # single ACT exp drains them (amortizes the ~352-cycle
                    # per-instruction ACT overhead).
                    for kh in range(2):
                        pts = [None] * 8
                        for ki in range(8):
                            kt = kh * 8 + ki
                            sps = ps_s.tile([128, 2, QCW], DT.float32, tag="ps_s")
                            for par in range(2):
                                prow = slice(par * D, par * D + D)
                                nc.tensor.matmul(
                                    sps[:, par, :],
                                    kT[prow, hp, kt * 128:(kt + 1) * 128],
                                    qT[prow, hp, q0:q0 + QCW],
                                    start=True,
                                    stop=True,
                                )
                            ptile = ptp.tile(
                                [128, 2, QCW], DT.bfloat16, tag=f"pt{ki}"
                            )
                            nc.scalar.activation(
                                out=ptile, in_=sps, func=AF.Exp,
                                scale=float(SCALE),
                            )
                            pts[ki] = ptile
                        for ki in range(8):
                            kt = kh * 8 + ki
                            for par in range(2):
                                nc.tensor.matmul(
                                    ovs[par],
                                    v_sb[:, kt, 2 * hp + par, :],
                                    pts[ki][:, par, :],
                                    start=(kt == 0),
                                    stop=(kt == ST - 1),
                                )
                    # normalize: O^T rows /= sums row (row index D).
                    # Drain psum to SBUF immediately (releases the ov bank in
                    # ~1.5us so the next round's attn@V isn't gated on the
                    # whole norm chain), then broadcast/reciprocal/multiply
                    # off the critical path.
                    for par in range(2):
                        ov = ovs[par]
                        sums = normp.tile([1, QCW], DT.float32, tag="sums")
                        nc.vector.tensor_copy(sums, ov[D:D + 1, :])
                        ovst = normp.tile([D, QCW], DT.float32, tag="ovst")
                        nc.vector.tensor_copy(ovst, ov[0:D, :])
                        bc = normp.tile([D, QCW], DT.float32, tag="bc")
                        nc.gpsimd.partition_broadcast(bc, sums)
                        nc.vector.reciprocal(bc, bc)
                        nc.vector.tensor_mul(
                            ot_sb[par * D:par * D + D, hp, q0:q0 + QCW],
                            ovst,
                            bc,
                        )

            if debug:
                for hp in range(NHP):
                    for c4 in range(4):
                        csl = slice(c4 * 512, (c4 + 1) * 512)
                        do = dbgp.tile([128, 512], DT.float32, tag="dbg")
                        nc.vector.tensor_copy(
                            do, ot_sb[:, hp, csl].bitcast(DT.float32))
                        nc.sync.dma_start(out=dbg_ot[:, hp, csl], in_=do)

            # ---------------- output projection ----------------
            for st in range(ST):
                ysb = youtp.tile([128, HID], DT.float32, tag="y")
                for nch in range(2):
                    yp = ps_p.tile([128, QCW], DT.float32, tag="ps1")
                    for hp in range(NHP):
                        nc.tensor.matmul(
                            yp,
                            ot_sb[:, hp, st * 128:(st + 1) * 128],
                            wo_sb[:, hp, nch * QCW:(nch + 1) * QCW],
                            start=(hp == 0),
                            stop=(hp == NHP - 1),
                        )
                    nc.vector.tensor_copy(ysb[:, nch * QCW:(nch + 1) * QCW], yp)
                nc.sync.dma_start(out=y[st * 128:(st + 1) * 128, :], in_=ysb)

    nc.finalize()
    return nc


def _get_nc():
    with _lock:
        if "nc" not in _cache:
            _cache["nc"] = _build()
        return _cache["nc"]


def _in_maps(q, k, v, Wq, Wk, Wv, Wo):
    maps = []
    for c in range(8):
        b, hf = c // 2, c % 2
        cs = slice(hf * E, (hf + 1) * E)
        maps.append({
            "xq": np.ascontiguousarray(q[b]),
            "xk": np.ascontiguousarray(k[b]),
            "xv": np.ascontiguousarray(v[b]),
            "wq": np.ascontiguousarray(Wq[:, cs]),
            "wk": np.ascontiguousarray(Wk[:, cs]),
            "wv": np.ascontiguousarray(Wv[:, cs]),
            "wo": np.ascontiguousarray(Wo[cs, :]),
        })
    return maps


def run(q, k, v, Wq, Wk, Wv, Wo, **spmd_kwargs):
    nc = _get_nc()
    res = run_bass_kernel_spmd(
        nc, _in_maps(q, k, v, Wq, Wk, Wv, Wo), core_ids=list(range(8)),
        **spmd_kwargs,
    )
    out = np.empty((B, S, HID), dtype=np.float32)
    for b in range(B):
        out[b] = res.results[2 * b]["y"] + res.results[2 * b + 1]["y"]
    return out, res


def kernel(q, k, v, Wq, Wk, Wv, Wo):
    out, _ = run(q, k, v, Wq, Wk, Wv, Wo)
    return out



# revision 6
# speedup vs baseline: 1.1918x; 1.1918x over previous
"""Multi-head attention kernel for Trainium2, 8 NeuronCores.

Problem: B=4, S=2048, HID=1024, H=16 heads, D=64.
  Q = q@Wq, K = k@Wk, V = v@Wv (reshaped to heads)
  O = softmax(Q K^T / sqrt(D)) V ;  out = O @ Wo

Sharding (hardcoded): core c handles batch b=c//2 and head-half hf=c%2
(8 of 16 heads via column-parallel Wq/Wk/Wv, row-parallel Wo).  Each core
returns a partial output [S, HID]; the host sums the two head-halves per
batch.

v2 design (single fused pipeline, ACT-exp paced):
  - Host pre-transposes q/k/v to [HID, S] and converts x + weights to
    bf16, so no on-chip transposes are needed: every projection matmul
    reads xT directly with the contraction dim on partitions.
  - The softmax exp on the scalar (ACT) engine is the hard floor
    (8 heads x 2048 x 2048 = 33.5M elem/core at ~1.2 G elem/s ~= 256us).
    The kernel is structured as 16 rounds (4 q-chunks x 4 head-pairs) of
    [scores pair -> exp -> attn@V], with the K/Q/Y projection matmuls
    emitted as small "filler units" into the PE slack inside the rounds
    so the PE never blocks the ACT engine.
  - V (+ a ones column for softmax row sums) and K et0/Q qc0 projections
    form the serial head; Y projection of the last q-chunk is the tail.
  - Normalization per round: one DVE reciprocal on a packed [2, 512]
    sums tile, gpsimd partition-broadcast, DVE multiply.
"""

import threading

import numpy as np

import concourse.bacc as bacc
import concourse.mybir as mybir
import concourse.tile as tile
from concourse.bass_utils import run_bass_kernel_spmd

DT = mybir.dt
AF = mybir.ActivationFunctionType

B, S, HID, H = 4, 2048, 1024, 16
D = HID // H               # 64
E = 512                    # local hidden (8 heads)
HLOC = 8                   # heads per core
NHP = 4                    # head pairs per core
SC = 4                     # s-chunks of 512
ST = 16                    # s-tiles of 128
CB = 8                     # contraction blocks of 128 (over HID)
ET = 4                     # e-tiles of 128 in Q^T/K^T
QCW = 512                  # q-chunk width
VW = D + 2                 # V row width: 64 data + ones col + pad (4B align)
SCALE = 1.0 / np.sqrt(np.float32(D))   # 0.125

_lock = threading.Lock()
_cache = {}


def _build():
    nc = bacc.Bacc(None)
    xqT = nc.declare_dram_parameter("xqT", [HID, S], DT.bfloat16, isOutput=False)
    xkT = nc.declare_dram_parameter("xkT", [HID, S], DT.bfloat16, isOutput=False)
    xvT = nc.declare_dram_parameter("xvT", [HID, S], DT.bfloat16, isOutput=False)
    wq = nc.declare_dram_parameter("wq", [HID, E], DT.bfloat16, isOutput=False)
    wk = nc.declare_dram_parameter("wk", [HID, E], DT.bfloat16, isOutput=False)
    wv = nc.declare_dram_parameter("wv", [HID, E], DT.bfloat16, isOutput=False)
    wo = nc.declare_dram_parameter("wo", [E, HID], DT.bfloat16, isOutput=False)
    y = nc.declare_dram_parameter("y", [S, HID], DT.float32, isOutput=True)

    with tile.TileContext(nc) as tc:
        with (
            tc.tile_pool(name="wpool", bufs=1) as wpool,
            tc.tile_pool(name="xpool", bufs=1) as xpool,
            tc.tile_pool(name="xqpool", bufs=1) as xqpool,
            tc.tile_pool(name="qkv", bufs=1) as qkvp,
            tc.tile_pool(name="pt", bufs=3) as ptp,
            tc.tile_pool(name="norm", bufs=2) as normp,
            tc.tile_pool(name="yout", bufs=2) as youtp,
            tc.tile_pool(name="ps_proj", bufs=2, space="PSUM") as ps_proj,
            tc.tile_pool(name="ps_s", bufs=2, space="PSUM") as ps_s,
            tc.tile_pool(name="ps_ov", bufs=1, space="PSUM") as ps_ov,
        ):
            # ---- weights (all resident) ----
            wq_sb = wpool.tile([128, CB, E], DT.bfloat16, tag="wq")
            wk_sb = wpool.tile([128, CB, E], DT.bfloat16, tag="wk")
            wv_sb = wpool.tile([128, CB, E], DT.bfloat16, tag="wv")
            for cb in range(CB):
                nc.sync.dma_start(out=wv_sb[:, cb, :], in_=wv[cb * 128:(cb + 1) * 128, :])
            for cb in range(CB):
                nc.sync.dma_start(out=wk_sb[:, cb, :], in_=wk[cb * 128:(cb + 1) * 128, :])
            for cb in range(CB):
                nc.sync.dma_start(out=wq_sb[:, cb, :], in_=wq[cb * 128:(cb + 1) * 128, :])
            wo_sb = wpool.tile([128, NHP, HID], DT.bfloat16, tag="wo")
            for eb in range(NHP):
                nc.sync.dma_start(out=wo_sb[:, eb, :], in_=wo[eb * 128:(eb + 1) * 128, :])

            # ---- x inputs ----
            xv_sb = xpool.tile([128, CB, S], DT.bfloat16, tag="xv")
            for cb in range(CB):
                nc.sync.dma_start(out=xv_sb[:, cb, :], in_=xvT[cb * 128:(cb + 1) * 128, :])
            xk_sb = xpool.tile([128, CB, S], DT.bfloat16, tag="xk")
            for cb in range(CB):
                nc.sync.dma_start(out=xk_sb[:, cb, :], in_=xkT[cb * 128:(cb + 1) * 128, :])

            xq_tiles = {}

            def q_prep(qc):
                xq_sb = xqpool.tile([128, CB, QCW], DT.bfloat16, tag="xq")
                for cb in range(CB):
                    nc.sync.dma_start(
                        out=xq_sb[:, cb, :],
                        in_=xqT[cb * 128:(cb + 1) * 128, qc * QCW:(qc + 1) * QCW],
                    )
                xq_tiles[qc] = xq_sb

            q_prep(0)

            # ---- persistent SBUF tensors ----
            qT = qkvp.tile([128, ET, S], DT.bfloat16, tag="qT")
            kT = qkvp.tile([128, ET, S], DT.bfloat16, tag="kT")
            v_sb = qkvp.tile([128, ST, HLOC, VW], DT.bfloat16, tag="v")
            nc.vector.memset(v_sb[:, :, :, D:D + 1], 1.0)
            ot_sb = qkvp.tile([128, NHP, S], DT.bfloat16, tag="ot")

            # ---- projection unit emitters ----
            def v_unit(st):
                pp = ps_proj.tile([128, E], DT.float32, tag="pp")
                for cb in range(CB):
                    nc.tensor.matmul(
                        pp,
                        xv_sb[:, cb, st * 128:(st + 1) * 128],
                        wv_sb[:, cb, :],
                        start=(cb == 0),
                        stop=(cb == CB - 1),
                    )
                nc.vector.tensor_copy(
                    v_sb[:, st, :, 0:D],
                    pp.rearrange("p (h d) -> p h d", h=HLOC),
                )

            def k_unit(et, sc):
                pp = ps_proj.tile([128, QCW], DT.float32, tag="pp")
                for cb in range(CB):
                    nc.tensor.matmul(
                        pp,
                        wk_sb[:, cb, et * 128:(et + 1) * 128],
                        xk_sb[:, cb, sc * QCW:(sc + 1) * QCW],
                        start=(cb == 0),
                        stop=(cb == CB - 1),
                    )
                nc.vector.tensor_copy(kT[:, et, sc * QCW:(sc + 1) * QCW], pp)

            def q_unit(qc, et):
                xq_sb = xq_tiles[qc]
                pp = ps_proj.tile([128, QCW], DT.float32, tag="pp")
                for cb in range(CB):
                    nc.tensor.matmul(
                        pp,
                        wq_sb[:, cb, et * 128:(et + 1) * 128],
                        xq_sb[:, cb, :],
                        start=(cb == 0),
                        stop=(cb == CB - 1),
                    )
                nc.vector.tensor_copy(qT[:, et, qc * QCW:(qc + 1) * QCW], pp)

            y_tiles = {}

            def y_unit(st, nch):
                if nch == 0:
                    y_tiles[st] = youtp.tile(
                        [128, HID], DT.float32, tag="y", name=f"ysb{st}")
                ysb = y_tiles[st]
                yp = ps_proj.tile([128, QCW], DT.float32, tag="pp")
                for hp in range(NHP):
                    nc.tensor.matmul(
                        yp,
                        ot_sb[:, hp, st * 128:(st + 1) * 128],
                        wo_sb[:, hp, nch * QCW:(nch + 1) * QCW],
                        start=(hp == 0),
                        stop=(hp == NHP - 1),
                    )
                nc.vector.tensor_copy(ysb[:, nch * QCW:(nch + 1) * QCW], yp)
                if nch == 1:
                    nc.sync.dma_start(out=y[st * 128:(st + 1) * 128, :], in_=ysb)

            # ---- serial head: V, K et0, Q qc0 (all et) ----
            for st in range(ST):
                v_unit(st)
            for sc in range(SC):
                k_unit(0, sc)
            for et in range(ET):
                q_unit(0, et)

            # ---- filler schedule: units pumped into round PE slack ----
            # round r = 4*qc + hp.  Readiness: K et needed by round hp=et of
            # qc0; Q(qc,*) by round 4*qc; Y(st in 4qc..) after round 4qc+3.
            fillers = {
                0: [lambda: q_prep(1)] + [
                    (lambda sc: (lambda: k_unit(1, sc)))(sc) for sc in range(SC)],
                1: [(lambda sc: (lambda: k_unit(2, sc)))(sc) for sc in range(SC)],
                2: [(lambda sc: (lambda: k_unit(3, sc)))(sc) for sc in range(SC)],
                3: [(lambda et: (lambda: q_unit(1, et)))(et) for et in range(ET)],
                4: [lambda: q_prep(2)] + [
                    (lambda a: (lambda: y_unit(0, a)))(a) for a in range(2)] + [
                    (lambda a: (lambda: y_unit(1, a)))(a) for a in range(2)],
                5: [(lambda a: (lambda: y_unit(2, a)))(a) for a in range(2)] + [
                    (lambda a: (lambda: y_unit(3, a)))(a) for a in range(2)],
                6: [(lambda et: (lambda: q_unit(2, et)))(et) for et in range(ET)],
                7: [lambda: q_prep(3)],
                8: [lambda: y_unit(4, 0), lambda: y_unit(4, 1),
                    lambda: y_unit(5, 0), lambda: y_unit(5, 1)],
                9: [lambda: y_unit(6, 0), lambda: y_unit(6, 1),
                    lambda: y_unit(7, 0), lambda: y_unit(7, 1)],
                10: [(lambda et: (lambda: q_unit(3, et)))(et) for et in range(ET)],
                11: [],
                12: [lambda: y_unit(8, 0), lambda: y_unit(8, 1),
                     lambda: y_unit(9, 0), lambda: y_unit(9, 1)],
                13: [lambda: y_unit(10, 0), lambda: y_unit(10, 1),
                     lambda: y_unit(11, 0), lambda: y_unit(11, 1)],
                14: [],
                15: [],
            }

            # ---- attention rounds ----
            for qc in range(SC):
                q0 = qc * QCW
                for hp in range(NHP):
                    r = 4 * qc + hp
                    pending = list(fillers.get(r, ()))
                    # spread filler units across the round's k-tiles
                    pump_at = {}
                    if pending:
                        step = max(1, ST // len(pending))
                        for i in range(len(pending)):
                            pump_at.setdefault(min(2 + i * step, ST - 1), []).append(i)
                    ovs = [
                        ps_ov.tile([D + 1, QCW], DT.float32, tag=f"ov{par}",
                                   name=f"ov{qc}_{hp}_{par}")
                        for par in range(2)
                    ]
                    for kt in range(ST):
                        sps = ps_s.tile([128, 2, QCW], DT.float32, tag="ps_s")
                        for par in range(2):
                            prow = slice(par * D, par * D + D)
                            nc.tensor.matmul(
                                sps[:, par, :],
                                kT[prow, hp, kt * 128:(kt + 1) * 128],
                                qT[prow, hp, q0:q0 + QCW],
                                start=True,
                                stop=True,
                            )
                        ptile = ptp.tile([128, 2, QCW], DT.bfloat16, tag="pt")
                        nc.scalar.activation(
                            out=ptile, in_=sps, func=AF.Exp, scale=float(SCALE),
                        )
                        for par in range(2):
                            nc.tensor.matmul(
                                ovs[par],
                                v_sb[:, kt, 2 * hp + par, 0:D + 1],
                                ptile[:, par, :],
                                start=(kt == 0),
                                stop=(kt == ST - 1),
                            )
                        for i in pump_at.get(kt, ()):
                            pending[i]()

                    # normalization: O^T rows /= sums row (psum row D).
                    # par sums live at partitions 0 and 32 (legal AP bases);
                    # one FD-bound reciprocal covers both.
                    sums = normp.tile([33, QCW], DT.float32, tag="sums")
                    ovsts = []
                    for par in range(2):
                        nc.vector.tensor_copy(
                            sums[32 * par:32 * par + 1, :], ovs[par][D:D + 1, :])
                        ovst = normp.tile([D, QCW], DT.float32, tag=f"ovst{par}")
                        nc.vector.tensor_copy(ovst, ovs[par][0:D, :])
                        ovsts.append(ovst)
                    nc.vector.reciprocal(sums, sums)
                    rrow = normp.tile([1, QCW], DT.float32, tag="rrow")
                    nc.vector.tensor_copy(rrow, sums[32:33, :])
                    for par in range(2):
                        src = sums[0:1, :] if par == 0 else rrow
                        bc = normp.tile([D, QCW], DT.float32, tag="bc")
                        nc.gpsimd.partition_broadcast(bc, src)
                        nc.vector.tensor_mul(
                            ot_sb[par * D:par * D + D, hp, q0:q0 + QCW],
                            ovsts[par],
                            bc,
                        )

            # ---- tail: Y projection of the last q-chunk ----
            for st in range(12, ST):
                y_unit(st, 0)
                y_unit(st, 1)

    nc.finalize()
    return nc


def _get_nc():
    with _lock:
        if "nc" not in _cache:
            _cache["nc"] = _build()
        return _cache["nc"]


def _in_maps(q, k, v, Wq, Wk, Wv, Wo):
    import ml_dtypes

    bf16 = ml_dtypes.bfloat16
    xT = {}
    for b in range(B):
        xT[b] = tuple(
            np.ascontiguousarray(t[b].astype(bf16).T) for t in (q, k, v)
        )
    w_bf = [
        (np.ascontiguousarray(W[:, hf * E:(hf + 1) * E].astype(bf16)) if W is not Wo
         else np.ascontiguousarray(W[hf * E:(hf + 1) * E, :].astype(bf16)))
        for hf in range(2) for W in (Wq, Wk, Wv, Wo)
    ]
    maps = []
    for c in range(8):
        b, hf = c // 2, c % 2
        qt, kt, vt = xT[b]
        wqc, wkc, wvc, woc = w_bf[hf * 4:(hf + 1) * 4]
        maps.append({
            "xqT": qt,
            "xkT": kt,
            "xvT": vt,
            "wq": wqc,
            "wk": wkc,
            "wv": wvc,
            "wo": woc,
        })
    return maps


def run(q, k, v, Wq, Wk, Wv, Wo, **spmd_kwargs):
    nc = _get_nc()
    res = run_bass_kernel_spmd(
        nc, _in_maps(q, k, v, Wq, Wk, Wv, Wo), core_ids=list(range(8)),
        **spmd_kwargs,
    )
    out = np.empty((B, S, HID), dtype=np.float32)
    for b in range(B):
        out[b] = res.results[2 * b]["y"] + res.results[2 * b + 1]["y"]
    return out, res


def kernel(q, k, v, Wq, Wk, Wv, Wo):
    out, _ = run(q, k, v, Wq, Wk, Wv, Wo)
    return out


# revision 8
# speedup vs baseline: 1.2443x; 1.0441x over previous
"""Multi-head attention kernel for Trainium2, 8 NeuronCores.

Problem: B=4, S=2048, HID=1024, H=16 heads, D=64.
  Q = q@Wq, K = k@Wk, V = v@Wv (reshaped to heads)
  O = softmax(Q K^T / sqrt(D)) V ;  out = O @ Wo

Sharding (hardcoded): core c handles batch b=c//2 and head-half hf=c%2
(8 of 16 heads via column-parallel Wq/Wk/Wv, row-parallel Wo).  Each core
returns a partial output [S, HID]; the host sums the two head-halves per
batch.

v3 design (single fused pipeline, ACT-exp paced):
  - Host pre-transposes q/k/v to [HID, S] and converts x + weights to
    bf16: no on-chip transposes; every projection matmul reads xT with
    the contraction dim on partitions.
  - The softmax exp on the scalar (ACT) engine is the hard floor
    (8 heads x 2048 x 2048 = 33.5M elem/core at ~1.2 G elem/s ~= 256us).
    Kernel = 16 rounds (4 q-chunks x 4 head-pairs) of
    [scores pair -> exp -> attn@V(2 k-tiles behind)], with K/Q/Y
    projection matmuls drip-fed 2-at-a-time into the per-k-tile PE slack
    via generators so the in-order PE queue never makes ACT wait.
  - V (+ones column for row sums) then K et0 then Q qc0 form the serial
    head, DMA-chunked so compute starts ~6us in; Y of the last q-chunk
    is the tail.
  - Normalization per round: packed DVE reciprocal ([33,512], rows 0/32),
    gpsimd partition-broadcast, DVE multiply; attn@V lags scores by 2
    k-tiles so psum-release latency never blocks the exp stream.
"""

import threading

import numpy as np

import concourse.bacc as bacc
import concourse.mybir as mybir
import concourse.tile as tile
from concourse.bass_utils import run_bass_kernel_spmd

DT = mybir.dt
AF = mybir.ActivationFunctionType

B, S, HID, H = 4, 2048, 1024, 16
D = HID // H               # 64
E = 512                    # local hidden (8 heads)
HLOC = 8                   # heads per core
NHP = 4                    # head pairs per core
SC = 4                     # s-chunks of 512
ST = 16                    # s-tiles of 128
CB = 8                     # contraction blocks of 128 (over HID)
ET = 4                     # e-tiles of 128 in Q^T/K^T
QCW = 512                  # q-chunk width
VW = D + 2                 # V row width: 64 data + ones col + pad (4B align)
SCALE = 1.0 / np.sqrt(np.float32(D))   # 0.125

_lock = threading.Lock()
_cache = {}


def _build():
    nc = bacc.Bacc(None)
    xqT = nc.declare_dram_parameter("xqT", [HID, S], DT.bfloat16, isOutput=False)
    xkT = nc.declare_dram_parameter("xkT", [HID, S], DT.bfloat16, isOutput=False)
    xvT = nc.declare_dram_parameter("xvT", [HID, S], DT.bfloat16, isOutput=False)
    wq = nc.declare_dram_parameter("wq", [HID, E], DT.bfloat16, isOutput=False)
    wk = nc.declare_dram_parameter("wk", [HID, E], DT.bfloat16, isOutput=False)
    wv = nc.declare_dram_parameter("wv", [HID, E], DT.bfloat16, isOutput=False)
    wo = nc.declare_dram_parameter("wo", [E, HID], DT.bfloat16, isOutput=False)
    y = nc.declare_dram_parameter("y", [S, HID], DT.float32, isOutput=True)

    with tile.TileContext(nc) as tc:
        with (
            tc.tile_pool(name="wpool", bufs=1) as wpool,
            tc.tile_pool(name="xpool", bufs=1) as xpool,
            tc.tile_pool(name="xqpool", bufs=1) as xqpool,
            tc.tile_pool(name="qkv", bufs=1) as qkvp,
            tc.tile_pool(name="pt", bufs=3) as ptp,
            tc.tile_pool(name="norm", bufs=2) as normp,
            tc.tile_pool(name="yout", bufs=2) as youtp,
            tc.tile_pool(name="ps_proj", bufs=2, space="PSUM") as ps_proj,
            tc.tile_pool(name="ps_s", bufs=2, space="PSUM") as ps_s,
            tc.tile_pool(name="ps_ov", bufs=1, space="PSUM") as ps_ov,
        ):
            # ---- DMAs, ordered by first use: wv, xv, wk, xk, wq, xq0, wo.
            # x tensors are chunked (cb, sc) so projections start early.
            wv_sb = wpool.tile([128, CB, E], DT.bfloat16, tag="wv")
            for cb in range(CB):
                nc.sync.dma_start(out=wv_sb[:, cb, :], in_=wv[cb * 128:(cb + 1) * 128, :])
            xv_sb = xpool.tile([128, CB, S], DT.bfloat16, tag="xv")
            for sc in range(SC):
                for cb in range(CB):
                    nc.sync.dma_start(
                        out=xv_sb[:, cb, sc * QCW:(sc + 1) * QCW],
                        in_=xvT[cb * 128:(cb + 1) * 128, sc * QCW:(sc + 1) * QCW],
                    )
            wk_sb = wpool.tile([128, CB, E], DT.bfloat16, tag="wk")
            for cb in range(CB):
                nc.sync.dma_start(out=wk_sb[:, cb, :], in_=wk[cb * 128:(cb + 1) * 128, :])
            xk_sb = xpool.tile([128, CB, S], DT.bfloat16, tag="xk")
            for sc in range(SC):
                for cb in range(CB):
                    nc.sync.dma_start(
                        out=xk_sb[:, cb, sc * QCW:(sc + 1) * QCW],
                        in_=xkT[cb * 128:(cb + 1) * 128, sc * QCW:(sc + 1) * QCW],
                    )
            wq_sb = wpool.tile([128, CB, E], DT.bfloat16, tag="wq")
            for cb in range(CB):
                nc.sync.dma_start(out=wq_sb[:, cb, :], in_=wq[cb * 128:(cb + 1) * 128, :])

            xq_tiles = {}

            def q_prep(qc):
                xq_sb = xqpool.tile([128, CB, QCW], DT.bfloat16, tag="xq",
                                    name=f"xq{qc}")
                for cb in range(CB):
                    nc.sync.dma_start(
                        out=xq_sb[:, cb, :],
                        in_=xqT[cb * 128:(cb + 1) * 128, qc * QCW:(qc + 1) * QCW],
                    )
                xq_tiles[qc] = xq_sb

            q_prep(0)
            wo_sb = wpool.tile([128, NHP, HID], DT.bfloat16, tag="wo")
            for eb in range(NHP):
                nc.sync.dma_start(out=wo_sb[:, eb, :], in_=wo[eb * 128:(eb + 1) * 128, :])

            # ---- persistent SBUF tensors ----
            qT = qkvp.tile([128, ET, S], DT.bfloat16, tag="qT")
            kT = qkvp.tile([128, ET, S], DT.bfloat16, tag="kT")
            v_sb = qkvp.tile([128, ST, HLOC, VW], DT.bfloat16, tag="v")
            nc.vector.memset(v_sb[:, :, :, D:D + 1], 1.0)
            ot_sb = qkvp.tile([128, NHP, S], DT.bfloat16, tag="ot")

            # preload the exp table set (~2.7us) during the head
            warm = normp.tile([1, 8], DT.float32, tag="warm")
            nc.vector.memset(warm, 0.0)
            nc.scalar.activation(out=warm, in_=warm, func=AF.Exp)

            # ---- projection units (generators: ~2 matmuls per step) ----
            def v_unit(st):
                pp = ps_proj.tile([128, E], DT.float32, tag="pp", name=f"vp{st}")
                for cb in range(CB):
                    nc.tensor.matmul(
                        pp,
                        xv_sb[:, cb, st * 128:(st + 1) * 128],
                        wv_sb[:, cb, :],
                        start=(cb == 0),
                        stop=(cb == CB - 1),
                    )
                nc.vector.tensor_copy(
                    v_sb[:, st, :, 0:D],
                    pp.rearrange("p (h d) -> p h d", h=HLOC),
                )

            def k_unit(et, sc):
                pp = ps_proj.tile([128, QCW], DT.float32, tag="pp",
                                  name=f"kp{et}_{sc}")
                for cb in range(CB):
                    nc.tensor.matmul(
                        pp,
                        wk_sb[:, cb, et * 128:(et + 1) * 128],
                        xk_sb[:, cb, sc * QCW:(sc + 1) * QCW],
                        start=(cb == 0),
                        stop=(cb == CB - 1),
                    )
                    if cb % 2 == 1 and cb < CB - 1:
                        yield
                nc.vector.tensor_copy(kT[:, et, sc * QCW:(sc + 1) * QCW], pp)

            def q_unit(qc, et):
                xq_sb = xq_tiles[qc]
                pp = ps_proj.tile([128, QCW], DT.float32, tag="pp",
                                  name=f"qp{qc}_{et}")
                for cb in range(CB):
                    nc.tensor.matmul(
                        pp,
                        wq_sb[:, cb, et * 128:(et + 1) * 128],
                        xq_sb[:, cb, :],
                        start=(cb == 0),
                        stop=(cb == CB - 1),
                    )
                    if cb % 2 == 1 and cb < CB - 1:
                        yield
                nc.vector.tensor_copy(qT[:, et, qc * QCW:(qc + 1) * QCW], pp)

            y_tiles = {}

            def y_unit(st, nch):
                if nch == 0:
                    y_tiles[st] = youtp.tile(
                        [128, HID], DT.float32, tag="y", name=f"ysb{st}")
                ysb = y_tiles[st]
                yp = ps_proj.tile([128, QCW], DT.float32, tag="pp",
                                  name=f"yp{st}_{nch}")
                for hp in range(NHP):
                    nc.tensor.matmul(
                        yp,
                        ot_sb[:, hp, st * 128:(st + 1) * 128],
                        wo_sb[:, hp, nch * QCW:(nch + 1) * QCW],
                        start=(hp == 0),
                        stop=(hp == NHP - 1),
                    )
                    if hp == 1:
                        yield
                nc.vector.tensor_copy(ysb[:, nch * QCW:(nch + 1) * QCW], yp)
                if nch == 1:
                    nc.sync.dma_start(out=y[st * 128:(st + 1) * 128, :], in_=ysb)

            def dma_unit(fn):
                fn()
                return
                yield  # make it a generator

            # ---- serial head: V, K et0, Q qc0 (all et) ----
            for st in range(ST):
                v_unit(st)
            for sc in range(SC):
                for _ in k_unit(0, sc):
                    pass
            for et in range(ET):
                for _ in q_unit(0, et):
                    pass

            # ---- filler schedule: one generator-step (~2 MMs) per k-tile.
            # round r = 4*qc + hp.  K et by round hp=et; Q(qc) by round
            # 4*qc; Y(qc) one full round after norm(qc,hp3) (cushion).
            fillers = {
                0: [dma_unit(lambda: q_prep(1))] + [k_unit(1, sc) for sc in range(SC)],
                1: [k_unit(2, sc) for sc in range(SC)],
                2: [k_unit(3, sc) for sc in range(SC)],
                3: [q_unit(1, et) for et in range(ET)],
                4: [dma_unit(lambda: q_prep(2))],
                5: [y_unit(st, a) for st in range(0, 4) for a in range(2)],
                6: [q_unit(2, et) for et in range(ET)],
                7: [dma_unit(lambda: q_prep(3))],
                8: [],
                9: [y_unit(st, a) for st in range(4, 8) for a in range(2)],
                10: [q_unit(3, et) for et in range(ET)],
                11: [],
                12: [],
                13: [y_unit(st, a) for st in range(8, 12) for a in range(2)],
                14: [],
                15: [],
            }

            # ---- attention rounds ----
            for qc in range(SC):
                q0 = qc * QCW
                for hp in range(NHP):
                    r = 4 * qc + hp
                    pending = list(fillers.get(r, ()))
                    ovs = [
                        ps_ov.tile([D + 1, QCW], DT.float32, tag=f"ov{par}",
                                   name=f"ov{qc}_{hp}_{par}")
                        for par in range(2)
                    ]
                    ptq = []  # (kt, ptile) awaiting attn@V, 2 k-tiles behind

                    def attn_v(kt, ptile):
                        for par in range(2):
                            nc.tensor.matmul(
                                ovs[par],
                                v_sb[:, kt, 2 * hp + par, 0:D + 1],
                                ptile[:, par, :],
                                start=(kt == 0),
                                stop=(kt == ST - 1),
                            )

                    for kt in range(ST):
                        sps = ps_s.tile([128, 2, QCW], DT.float32, tag="ps_s")
                        for par in range(2):
                            prow = slice(par * D, par * D + D)
                            nc.tensor.matmul(
                                sps[:, par, :],
                                kT[prow, hp, kt * 128:(kt + 1) * 128],
                                qT[prow, hp, q0:q0 + QCW],
                                start=True,
                                stop=True,
                            )
                        ptile = ptp.tile([128, 2, QCW], DT.bfloat16, tag="pt")
                        nc.scalar.activation(
                            out=ptile, in_=sps, func=AF.Exp, scale=float(SCALE),
                        )
                        ptq.append((kt, ptile))
                        if kt >= 2:
                            attn_v(*ptq.pop(0))
                        # drip-feed ~2 filler matmuls into the PE slack
                        if pending:
                            try:
                                next(pending[0])
                            except StopIteration:
                                pending.pop(0)
                    while ptq:
                        attn_v(*ptq.pop(0))
                    for g in pending:  # finish any leftover units
                        for _ in g:
                            pass

                    # normalization: O^T rows /= sums row (psum row D).
                    # par sums at partitions 0 and 32 (legal AP bases);
                    # one FD-bound reciprocal covers both.
                    sums = normp.tile([33, QCW], DT.float32, tag="sums")
                    ovsts = []
                    for par in range(2):
                        nc.vector.tensor_copy(
                            sums[32 * par:32 * par + 1, :], ovs[par][D:D + 1, :])
                        ovst = normp.tile([D, QCW], DT.float32, tag=f"ovst{par}")
                        nc.vector.tensor_copy(ovst, ovs[par][0:D, :])
                        ovsts.append(ovst)
                    nc.vector.reciprocal(sums, sums)
                    rrow = normp.tile([1, QCW], DT.float32, tag="rrow")
                    nc.vector.tensor_copy(rrow, sums[32:33, :])
                    for par in range(2):
                        src = sums[0:1, :] if par == 0 else rrow
                        bc = normp.tile([D, QCW], DT.float32, tag="bc")
                        nc.gpsimd.partition_broadcast(bc, src)
                        nc.vector.tensor_mul(
                            ot_sb[par * D:par * D + D, hp, q0:q0 + QCW],
                            ovsts[par],
                            bc,
                        )

            # ---- tail: Y projection of the last q-chunk ----
            for st in range(12, ST):
                for nch in range(2):
                    for _ in y_unit(st, nch):
                        pass

    nc.finalize()
    return nc


def _get_nc():
    with _lock:
        if "nc" not in _cache:
            _cache["nc"] = _build()
        return _cache["nc"]


def _in_maps(q, k, v, Wq, Wk, Wv, Wo):
    import ml_dtypes

    bf16 = ml_dtypes.bfloat16
    xT = {}
    for b in range(B):
        xT[b] = tuple(
            np.ascontiguousarray(t[b].astype(bf16).T) for t in (q, k, v)
        )
    w_bf = [
        (np.ascontiguousarray(W[:, hf * E:(hf + 1) * E].astype(bf16)) if W is not Wo
         else np.ascontiguousarray(W[hf * E:(hf + 1) * E, :].astype(bf16)))
        for hf in range(2) for W in (Wq, Wk, Wv, Wo)
    ]
    maps = []
    for c in range(8):
        b, hf = c // 2, c % 2
        qt, kt, vt = xT[b]
        wqc, wkc, wvc, woc = w_bf[hf * 4:(hf + 1) * 4]
        maps.append({
            "xqT": qt,
            "xkT": kt,
            "xvT": vt,
            "wq": wqc,
            "wk": wkc,
            "wv": wvc,
            "wo": woc,
        })
    return maps


def run(q, k, v, Wq, Wk, Wv, Wo, **spmd_kwargs):
    nc = _get_nc()
    res = run_bass_kernel_spmd(
        nc, _in_maps(q, k, v, Wq, Wk, Wv, Wo), core_ids=list(range(8)),
        **spmd_kwargs,
    )
    out = np.empty((B, S, HID), dtype=np.float32)
    for b in range(B):
        out[b] = res.results[2 * b]["y"] + res.results[2 * b + 1]["y"]
    return out, res


def kernel(q, k, v, Wq, Wk, Wv, Wo):
    out, _ = run(q, k, v, Wq, Wk, Wv, Wo)
    return out


# revision 25
# speedup vs baseline: 1.3557x; 1.0896x over previous
"""Multi-head attention kernel for Trainium2, 8 NeuronCores.

Problem: B=4, S=2048, HID=1024, H=16 heads, D=64.
  Q = q@Wq, K = k@Wk, V = v@Wv (reshaped to heads)
  O = softmax(Q K^T / sqrt(D)) V ;  out = O @ Wo

Sharding (hardcoded): core c handles batch b=c//2 and head-half hf=c%2
(8 of 16 heads via column-parallel Wq/Wk/Wv, row-parallel Wo).  Each core
returns a partial output [S, HID]; the host sums the two head-halves per
batch.

v5 design (single fused pipeline, ACT-exp paced):
  - Host pre-transposes q/k/v to [HID, S] and converts x + weights to
    bf16: no on-chip transposes; every projection matmul reads xT with
    the contraction dim on partitions.
  - The softmax exp on the scalar (ACT) engine is the hard floor
    (8 heads x 2048 x 2048 = 33.5M elem/core at ~1.06us per 1024-wide
    drain ~= 272us).  The kernel is one flat stream of 256 k-tile steps
    (16 rounds x 16 k-tiles) of [scores pair -> exp -> attn@V lagging 3
    steps], with K/Q/Y projection matmuls drip-fed ~2 per step from a
    global generator deque.  The fillers both hide the projection work
    inside the ACT-paced slack and keep the PE dense enough that the
    HAM clock gate stays at full rate.
  - DMA descriptor generation costs ~0.6us per contiguous line on the
    issuing engine queue, so x tensors use 8 one-line dma_starts each,
    spread across the sync/scalar/gpsimd queues to issue in parallel.
  - Normalization per round r is emitted 2 steps into round r+1 (right
    after attn@V(r,15)): ovst copies first (releases the
    single-buffered ov psum), packed reciprocal ([33,512], rows 0/32),
    gpsimd partition-broadcast, multiply.  Y(qc) fillers are placed so
    their hp3 matmul trails the qc's last norm by >=4 k-tiles.
"""

import threading

import numpy as np

import concourse.bacc as bacc
import concourse.mybir as mybir
import concourse.tile as tile
from concourse.bass_utils import run_bass_kernel_spmd

DT = mybir.dt
AF = mybir.ActivationFunctionType

B, S, HID, H = 4, 2048, 1024, 16
D = HID // H               # 64
E = 512                    # local hidden (8 heads)
HLOC = 8                   # heads per core
NHP = 4                    # head pairs per core
SC = 4                     # s-chunks of 512
ST = 16                    # s-tiles of 128
CB = 8                     # contraction blocks of 128 (over HID)
ET = 4                     # e-tiles of 128 in Q^T/K^T
QCW = 512                  # q-chunk width
VW = D + 2                 # V row width: 64 data + ones col + pad (4B align)
LAG = 4                    # attn@V trails scores/exp by this many k-tiles
SCALE = 1.0 / np.sqrt(np.float32(D))   # 0.125

_lock = threading.Lock()
_cache = {}


def _build():
    nc = bacc.Bacc(None)
    xqT = nc.declare_dram_parameter("xqT", [HID, S], DT.bfloat16, isOutput=False)
    xkT = nc.declare_dram_parameter("xkT", [HID, S], DT.bfloat16, isOutput=False)
    xvT = nc.declare_dram_parameter("xvT", [HID, S], DT.bfloat16, isOutput=False)
    wq = nc.declare_dram_parameter("wq", [HID, E], DT.bfloat16, isOutput=False)
    wk = nc.declare_dram_parameter("wk", [HID, E], DT.bfloat16, isOutput=False)
    wv = nc.declare_dram_parameter("wv", [HID, E], DT.bfloat16, isOutput=False)
    wo = nc.declare_dram_parameter("wo", [E, HID], DT.bfloat16, isOutput=False)
    y = nc.declare_dram_parameter("y", [S, HID], DT.float32, isOutput=True)

    with tile.TileContext(nc) as tc:
        with (
            tc.tile_pool(name="wpool", bufs=1) as wpool,
            tc.tile_pool(name="xpool", bufs=1) as xpool,
            tc.tile_pool(name="xqpool", bufs=1) as xqpool,
            tc.tile_pool(name="qkv", bufs=1) as qkvp,
            tc.tile_pool(name="pt", bufs=LAG + 2) as ptp,
            tc.tile_pool(name="norm2", bufs=2) as norm2p,
            tc.tile_pool(name="yout", bufs=1) as youtp,
            tc.tile_pool(name="ps_proj", bufs=2, space="PSUM") as ps_proj,
            tc.tile_pool(name="ps_s", bufs=2, space="PSUM") as ps_s,
            tc.tile_pool(name="ps_ov", bufs=1, space="PSUM") as ps_ov,
        ):
            # ---- DMAs: descriptor-gen spread across engine queues so
            # transfers overlap; x tensors in (cb, s-half) chunks so the
            # V/K projections start as soon as their half has landed.
            wv_sb = wpool.tile([128, CB, E], DT.bfloat16, tag="wv")
            nc.sync.dma_start(
                out=wv_sb, in_=wv.rearrange("(cb p) e -> p cb e", p=128))
            wk_sb = wpool.tile([128, CB, E], DT.bfloat16, tag="wk")
            nc.scalar.dma_start(
                out=wk_sb, in_=wk.rearrange("(cb p) e -> p cb e", p=128))
            xv_sb = xpool.tile([128, CB, S], DT.bfloat16, tag="xv")
            xk_sb = xpool.tile([128, CB, S], DT.bfloat16, tag="xk")
            for h in range(2):
                hs = slice(h * (S // 2), (h + 1) * (S // 2))
                for cb in range(CB):
                    nc.sync.dma_start(
                        out=xv_sb[:, cb, hs],
                        in_=xvT[cb * 128:(cb + 1) * 128, hs])
                for cb in range(CB):
                    nc.scalar.dma_start(
                        out=xk_sb[:, cb, hs],
                        in_=xkT[cb * 128:(cb + 1) * 128, hs])
            wq_sb = wpool.tile([128, CB, E], DT.bfloat16, tag="wq")
            nc.scalar.dma_start(
                out=wq_sb, in_=wq.rearrange("(cb p) e -> p cb e", p=128))

            xq_tiles = {}

            def q_prep(qc, eng=None):
                xq_sb = xqpool.tile([128, CB, QCW], DT.bfloat16, tag="xq",
                                    name=f"xq{qc}")
                (eng or nc.sync).dma_start(
                    out=xq_sb,
                    in_=xqT[:, qc * QCW:(qc + 1) * QCW].rearrange(
                        "(cb p) s -> p cb s", p=128),
                )
                xq_tiles[qc] = xq_sb

            q_prep(0, eng=nc.scalar)
            wo_sb = wpool.tile([128, NHP, HID], DT.bfloat16, tag="wo")
            nc.scalar.dma_start(
                out=wo_sb, in_=wo.rearrange("(eb p) n -> p eb n", p=128))

            # ---- persistent SBUF tensors ----
            qT = qkvp.tile([128, ET, S], DT.bfloat16, tag="qT")
            kT = qkvp.tile([128, ET, S], DT.bfloat16, tag="kT")
            v_sb = qkvp.tile([128, ST, HLOC, VW], DT.bfloat16, tag="v")
            nc.vector.memset(v_sb[:, :, :, D:D + 1], 1.0)
            ot_sb = qkvp.tile([128, NHP, S], DT.bfloat16, tag="ot")

            # preload the exp table set (~2.7us) during the head
            warm = norm2p.tile([1, 8], DT.float32, tag="warm")
            nc.vector.memset(warm, 0.0)
            nc.scalar.activation(out=warm, in_=warm, func=AF.Exp)

            # ---- projection units (generators: ~2 matmuls per step) ----
            def v_unit(st):
                pp = ps_proj.tile([128, E], DT.float32, tag="pp", name=f"vp{st}")
                for cb in range(CB):
                    nc.tensor.matmul(
                        pp,
                        xv_sb[:, cb, st * 128:(st + 1) * 128],
                        wv_sb[:, cb, :],
                        start=(cb == 0),
                        stop=(cb == CB - 1),
                    )
                nc.vector.tensor_copy(
                    v_sb[:, st, :, 0:D],
                    pp.rearrange("p (h d) -> p h d", h=HLOC),
                )

            def k_unit(et, sc):
                pp = ps_proj.tile([128, QCW], DT.float32, tag="pp",
                                  name=f"kp{et}_{sc}")
                for cb in range(CB):
                    nc.tensor.matmul(
                        pp,
                        wk_sb[:, cb, et * 128:(et + 1) * 128],
                        xk_sb[:, cb, sc * QCW:(sc + 1) * QCW],
                        start=(cb == 0),
                        stop=(cb == CB - 1),
                    )
                    if cb % 2 == 1 and cb < CB - 1:
                        yield
                nc.vector.tensor_copy(kT[:, et, sc * QCW:(sc + 1) * QCW], pp)

            def q_unit(qc, et):
                xq_sb = xq_tiles[qc]
                pp = ps_proj.tile([128, QCW], DT.float32, tag="pp",
                                  name=f"qp{qc}_{et}")
                for cb in range(CB):
                    nc.tensor.matmul(
                        pp,
                        wq_sb[:, cb, et * 128:(et + 1) * 128],
                        xq_sb[:, cb, :],
                        start=(cb == 0),
                        stop=(cb == CB - 1),
                    )
                    if cb % 2 == 1 and cb < CB - 1:
                        yield
                nc.vector.tensor_copy(qT[:, et, qc * QCW:(qc + 1) * QCW], pp)

            y_tiles = {}
            yp_tiles = {}

            def y_start(st, nch, n_hp):
                if nch == 0:
                    y_tiles[st] = youtp.tile(
                        [128, HID], DT.float32, tag="y", name=f"ysb{st}")
                yp = ps_proj.tile([128, QCW], DT.float32, tag="pp",
                                  name=f"yp{st}_{nch}")
                yp_tiles[(st, nch)] = yp
                for hp in range(n_hp):
                    nc.tensor.matmul(
                        yp,
                        ot_sb[:, hp, st * 128:(st + 1) * 128],
                        wo_sb[:, hp, nch * QCW:(nch + 1) * QCW],
                        start=(hp == 0),
                        stop=False,
                    )
                    if hp == 1:
                        yield

            def y_finish(st, nch, hp0):
                yp = yp_tiles.pop((st, nch))
                ysb = y_tiles[st]
                for hp in range(hp0, NHP):
                    nc.tensor.matmul(
                        yp,
                        ot_sb[:, hp, st * 128:(st + 1) * 128],
                        wo_sb[:, hp, nch * QCW:(nch + 1) * QCW],
                        start=False,
                        stop=(hp == NHP - 1),
                    )
                nc.vector.tensor_copy(ysb[:, nch * QCW:(nch + 1) * QCW], yp)
                if nch == 1:
                    nc.sync.dma_start(out=y[st * 128:(st + 1) * 128, :], in_=ysb)

            def y_unit(st, nch):
                yield from y_start(st, nch, 2)
                y_finish(st, nch, 2)

            def y_prefix(st, nch):
                yield from y_start(st, nch, 2)

            def dma_unit(fn):
                fn()
                return
                yield  # generator

            def skip(n):
                for _ in range(n):
                    yield

            junk_n = [0]

            def junk_unit(n):
                # dead score-shaped matmuls: keep the PE dense enough that
                # the HAM clock gate stays at full rate in thin rounds
                for i in range(n):
                    junk_n[0] += 1
                    jp = ps_s.tile([128, 2, QCW], DT.float32, tag="ps_s",
                                   name=f"junk{junk_n[0]}")
                    for par in range(2):
                        nc.tensor.matmul(
                            jp[:, par, :],
                            kT[par * D:par * D + D, 0, 0:128],
                            qT[par * D:par * D + D, 0, 0:QCW],
                            start=True,
                            stop=True,
                        )
                    yield

            # ---- serial head: V, K et0, Q qc0 et0..1 ----
            for st in range(ST):
                v_unit(st)
            for sc in range(SC):
                for _ in k_unit(0, sc):
                    pass
            for et in range(ET):
                for _ in q_unit(0, et):
                    pass

            # ---- filler schedule (deque order; ~1 step per k-tile).
            # K(et,sc) before round hp=et reaches k-tile 4*sc (JIT);
            # Q(qc,et) before round 4qc+et; Y(qc,..) with hp3 >=4 k-tiles
            # after norm(qc,hp3) which is emitted at round 4qc+4, kt2.
            fillers = {
                0: [dma_unit(lambda: q_prep(1)), k_unit(1, 0), k_unit(1, 1), k_unit(1, 2)],
                1: [k_unit(1, 3), k_unit(2, 0), k_unit(2, 1), k_unit(2, 2)],
                2: [k_unit(2, 3), k_unit(3, 0), k_unit(3, 1), k_unit(3, 2)],
                3: [k_unit(3, 3), q_unit(1, 0), q_unit(1, 1), q_unit(1, 2)],
                4: [q_unit(1, 3), dma_unit(lambda: q_prep(2)), junk_unit(8)],
                5: [y_unit(0, 0), y_unit(0, 1), y_unit(1, 0), q_unit(2, 0)],
                6: [y_unit(1, 1), y_unit(2, 0), y_unit(2, 1), q_unit(2, 1)],
                7: [y_unit(3, 0), y_unit(3, 1), q_unit(2, 2), q_unit(2, 3),
                    dma_unit(lambda: q_prep(3))],
                8: [q_unit(3, 0), junk_unit(8)],
                9: [y_unit(4, 0), y_unit(4, 1), y_unit(5, 0), q_unit(3, 1)],
                10: [y_unit(5, 1), y_unit(6, 0), y_unit(6, 1), q_unit(3, 2)],
                11: [y_unit(7, 0), y_unit(7, 1), q_unit(3, 3), junk_unit(3)],
                12: [junk_unit(10)],
                13: [y_unit(8, 0), y_unit(8, 1), y_unit(9, 0), y_unit(9, 1),
                     junk_unit(2)],
                14: [y_unit(10, 0), y_unit(10, 1), y_unit(11, 0), y_unit(11, 1),
                     junk_unit(2)],
                15: [junk_unit(7)],
            }
            pending = []

            # ---- attention: flat stream of 256 k-tile steps ----
            rounds = [(qc, hp) for qc in range(SC) for hp in range(NHP)]
            ov_tiles = {}
            ptq = []  # (r, kt, ptile)

            def attn_v(r, kt, ptile):
                qc, hp = rounds[r]
                if r not in ov_tiles:
                    ov_tiles[r] = [
                        ps_ov.tile([D + 1, QCW], DT.float32, tag=f"ov{par}",
                                   name=f"ov{r}_{par}")
                        for par in range(2)
                    ]
                ovs = ov_tiles[r]
                for par in range(2):
                    nc.tensor.matmul(
                        ovs[par],
                        v_sb[:, kt, 2 * hp + par, 0:D + 1],
                        ptile[:, par, :],
                        start=(kt == 0),
                        stop=(kt == ST - 1),
                    )

            norm_state = {}

            def norm_a(r):
                # copies only: releases the ov psum banks quickly and
                # keeps the DVE FIFO clear for filler drains
                ovs = ov_tiles.pop(r)
                ovsts = []
                for par in range(2):
                    ovst = norm2p.tile([D, QCW], DT.float32, tag=f"ovst{par}")
                    nc.vector.tensor_copy(ovst, ovs[par][0:D, :])
                    ovsts.append(ovst)
                sums = norm2p.tile([33, QCW], DT.float32, tag="sums")
                for par in range(2):
                    nc.vector.tensor_copy(
                        sums[32 * par:32 * par + 1, :], ovs[par][D:D + 1, :])
                norm_state[r] = (ovsts, sums)

            def norm_b(r):
                qc, hp = rounds[r]
                q0 = qc * QCW
                ovsts, sums = norm_state.pop(r)
                nc.vector.reciprocal(sums, sums)
                rrow = norm2p.tile([1, QCW], DT.float32, tag="rrow")
                nc.vector.tensor_copy(rrow, sums[32:33, :])
                for par in range(2):
                    src_ = sums[0:1, :] if par == 0 else rrow
                    bc = norm2p.tile([D, QCW], DT.float32, tag="bc")
                    nc.gpsimd.partition_broadcast(bc, src_)
                    nc.vector.tensor_mul(
                        ot_sb[par * D:par * D + D, hp, q0:q0 + QCW],
                        ovsts[par],
                        bc,
                    )

            for i in range(ST * len(rounds)):
                r, kt = divmod(i, ST)
                qc, hp = rounds[r]
                if kt == 0:
                    pending.extend(fillers.get(r, ()))
                sps = ps_s.tile([128, 2, QCW], DT.float32, tag="ps_s")
                for par in range(2):
                    prow = slice(par * D, par * D + D)
                    nc.tensor.matmul(
                        sps[:, par, :],
                        kT[prow, hp, kt * 128:(kt + 1) * 128],
                        qT[prow, hp, qc * QCW:(qc + 1) * QCW],
                        start=True,
                        stop=True,
                    )
                ptile = ptp.tile([128, 2, QCW], DT.bfloat16, tag="pt")
                nc.scalar.activation(
                    out=ptile, in_=sps, func=AF.Exp, scale=float(SCALE),
                )
                ptq.append((r, kt, ptile))
                if len(ptq) > LAG:
                    attn_v(*ptq.pop(0))
                if kt == LAG - 1 and r > 0:
                    norm_a(r - 1)
                if kt == LAG + 3 and r > 0:
                    norm_b(r - 1)
                # drip-feed ~2 filler matmuls into the PE slack
                if pending:
                    try:
                        next(pending[0])
                    except StopIteration:
                        pending.pop(0)

            while ptq:
                attn_v(*ptq.pop(0))
            for g in pending:
                for _ in g:
                    pass
            norm_a(15)
            # keep-warm bridge spanning the final norm chain: emitted
            # BEFORE the Y matmuls (whose weight-loads serialize on the
            # last norm multiply) so the PE queue isn't head-of-line
            # blocked and the HAM clock gate stays warm
            for _ in junk_unit(22):
                pass
            for nch in range(2):
                for _ in y_start(12, nch, 3):
                    pass
            norm_b(15)

            # ---- tail: finish Y of the last q-chunk ----
            y_finish(12, 0, 3)
            y_finish(12, 1, 3)
            for st in range(13, ST):
                for nch in range(2):
                    for _ in y_unit(st, nch):
                        pass

    nc.finalize()
    return nc


def _get_nc():
    with _lock:
        if "nc" not in _cache:
            _cache["nc"] = _build()
        return _cache["nc"]


def _in_maps(q, k, v, Wq, Wk, Wv, Wo):
    import ml_dtypes

    bf16 = ml_dtypes.bfloat16
    xT = {}
    for b in range(B):
        xT[b] = tuple(
            np.ascontiguousarray(t[b].astype(bf16).T) for t in (q, k, v)
        )
    w_bf = [
        (np.ascontiguousarray(W[:, hf * E:(hf + 1) * E].astype(bf16)) if W is not Wo
         else np.ascontiguousarray(W[hf * E:(hf + 1) * E, :].astype(bf16)))
        for hf in range(2) for W in (Wq, Wk, Wv, Wo)
    ]
    maps = []
    for c in range(8):
        b, hf = c // 2, c % 2
        qt, kt, vt = xT[b]
        wqc, wkc, wvc, woc = w_bf[hf * 4:(hf + 1) * 4]
        maps.append({
            "xqT": qt,
            "xkT": kt,
            "xvT": vt,
            "wq": wqc,
            "wk": wkc,
            "wv": wvc,
            "wo": woc,
        })
    return maps


def run(q, k, v, Wq, Wk, Wv, Wo, **spmd_kwargs):
    nc = _get_nc()
    res = run_bass_kernel_spmd(
        nc, _in_maps(q, k, v, Wq, Wk, Wv, Wo), core_ids=list(range(8)),
        **spmd_kwargs,
    )
    out = np.empty((B, S, HID), dtype=np.float32)
    for b in range(B):
        out[b] = res.results[2 * b]["y"] + res.results[2 * b + 1]["y"]
    return out, res


def kernel(q, k, v, Wq, Wk, Wv, Wo):
    out, _ = run(q, k, v, Wq, Wk, Wv, Wo)
    return out


# revision 26
# speedup vs baseline: 1.3807x; 1.0185x over previous
"""Multi-head attention kernel for Trainium2, 8 NeuronCores.

Problem: B=4, S=2048, HID=1024, H=16 heads, D=64.
  Q = q@Wq, K = k@Wk, V = v@Wv (reshaped to heads)
  O = softmax(Q K^T / sqrt(D)) V ;  out = O @ Wo

Sharding (hardcoded): core c handles batch b=c//2 and head-half hf=c%2
(8 of 16 heads via column-parallel Wq/Wk/Wv, row-parallel Wo).  Each core
returns a partial output [S, HID]; the host sums the two head-halves per
batch.

v5 design (single fused pipeline, ACT-exp paced):
  - Host pre-transposes q/k/v to [HID, S] and converts x + weights to
    bf16: no on-chip transposes; every projection matmul reads xT with
    the contraction dim on partitions.
  - The softmax exp on the scalar (ACT) engine is the hard floor
    (8 heads x 2048 x 2048 = 33.5M elem/core at ~1.06us per 1024-wide
    drain ~= 272us).  The kernel is one flat stream of 256 k-tile steps
    (16 rounds x 16 k-tiles) of [scores pair -> exp -> attn@V lagging 3
    steps], with K/Q/Y projection matmuls drip-fed ~2 per step from a
    global generator deque.  The fillers both hide the projection work
    inside the ACT-paced slack and keep the PE dense enough that the
    HAM clock gate stays at full rate.
  - DMA descriptor generation costs ~0.6us per contiguous line on the
    issuing engine queue, so x tensors use 8 one-line dma_starts each,
    spread across the sync/scalar/gpsimd queues to issue in parallel.
  - Normalization per round r is emitted 2 steps into round r+1 (right
    after attn@V(r,15)): ovst copies first (releases the
    single-buffered ov psum), packed reciprocal ([33,512], rows 0/32),
    gpsimd partition-broadcast, multiply.  Y(qc) fillers are placed so
    their hp3 matmul trails the qc's last norm by >=4 k-tiles.
"""

import threading

import numpy as np

import concourse.bacc as bacc
import concourse.mybir as mybir
import concourse.tile as tile
from concourse.bass_utils import run_bass_kernel_spmd

DT = mybir.dt
AF = mybir.ActivationFunctionType

B, S, HID, H = 4, 2048, 1024, 16
D = HID // H               # 64
E = 512                    # local hidden (8 heads)
HLOC = 8                   # heads per core
NHP = 4                    # head pairs per core
SC = 4                     # s-chunks of 512
ST = 16                    # s-tiles of 128
CB = 8                     # contraction blocks of 128 (over HID)
ET = 4                     # e-tiles of 128 in Q^T/K^T
QCW = 512                  # q-chunk width
VW = D + 2                 # V row width: 64 data + ones col + pad (4B align)
LAG = 4                    # attn@V trails scores/exp by this many k-tiles
SCALE = 1.0 / np.sqrt(np.float32(D))   # 0.125

_lock = threading.Lock()
_cache = {}


def _build():
    nc = bacc.Bacc(None)
    xqT = nc.declare_dram_parameter("xqT", [HID, S], DT.bfloat16, isOutput=False)
    xkT = nc.declare_dram_parameter("xkT", [HID, S], DT.bfloat16, isOutput=False)
    xvT = nc.declare_dram_parameter("xvT", [HID, S], DT.bfloat16, isOutput=False)
    wq = nc.declare_dram_parameter("wq", [HID, E], DT.bfloat16, isOutput=False)
    wk = nc.declare_dram_parameter("wk", [HID, E], DT.bfloat16, isOutput=False)
    wv = nc.declare_dram_parameter("wv", [HID, E], DT.bfloat16, isOutput=False)
    wo = nc.declare_dram_parameter("wo", [E, HID], DT.bfloat16, isOutput=False)
    y = nc.declare_dram_parameter("y", [S, HID], DT.float32, isOutput=True)

    with tile.TileContext(nc) as tc:
        with (
            tc.tile_pool(name="wpool", bufs=1) as wpool,
            tc.tile_pool(name="xpool", bufs=1) as xpool,
            tc.tile_pool(name="xqpool", bufs=1) as xqpool,
            tc.tile_pool(name="qkv", bufs=1) as qkvp,
            tc.tile_pool(name="pt", bufs=LAG + 2) as ptp,
            tc.tile_pool(name="norm2", bufs=2) as norm2p,
            tc.tile_pool(name="yout", bufs=2) as youtp,
            tc.tile_pool(name="ps_proj", bufs=2, space="PSUM") as ps_proj,
            tc.tile_pool(name="ps_s", bufs=2, space="PSUM") as ps_s,
            tc.tile_pool(name="ps_ov", bufs=1, space="PSUM") as ps_ov,
        ):
            # ---- DMAs: descriptor-gen spread across engine queues so
            # transfers overlap; x tensors in (cb, s-half) chunks so the
            # V/K projections start as soon as their half has landed.
            wv_sb = wpool.tile([128, CB, E], DT.bfloat16, tag="wv")
            nc.sync.dma_start(
                out=wv_sb, in_=wv.rearrange("(cb p) e -> p cb e", p=128))
            wk_sb = wpool.tile([128, CB, E], DT.bfloat16, tag="wk")
            nc.scalar.dma_start(
                out=wk_sb, in_=wk.rearrange("(cb p) e -> p cb e", p=128))
            xv_sb = xpool.tile([128, CB, S], DT.bfloat16, tag="xv")
            xk_sb = xpool.tile([128, CB, S], DT.bfloat16, tag="xk")
            for h in range(2):
                hs = slice(h * (S // 2), (h + 1) * (S // 2))
                for cb in range(CB):
                    nc.sync.dma_start(
                        out=xv_sb[:, cb, hs],
                        in_=xvT[cb * 128:(cb + 1) * 128, hs])
                for cb in range(CB):
                    nc.scalar.dma_start(
                        out=xk_sb[:, cb, hs],
                        in_=xkT[cb * 128:(cb + 1) * 128, hs])
            wq_sb = wpool.tile([128, CB, E], DT.bfloat16, tag="wq")
            nc.scalar.dma_start(
                out=wq_sb, in_=wq.rearrange("(cb p) e -> p cb e", p=128))

            xq_tiles = {}

            def q_prep(qc, eng=None):
                xq_sb = xqpool.tile([128, CB, QCW], DT.bfloat16, tag="xq",
                                    name=f"xq{qc}")
                (eng or nc.sync).dma_start(
                    out=xq_sb,
                    in_=xqT[:, qc * QCW:(qc + 1) * QCW].rearrange(
                        "(cb p) s -> p cb s", p=128),
                )
                xq_tiles[qc] = xq_sb

            q_prep(0, eng=nc.scalar)
            wo_sb = wpool.tile([128, NHP, HID], DT.bfloat16, tag="wo")
            nc.scalar.dma_start(
                out=wo_sb, in_=wo.rearrange("(eb p) n -> p eb n", p=128))

            # ---- persistent SBUF tensors ----
            qT = qkvp.tile([128, ET, S], DT.bfloat16, tag="qT")
            kT = qkvp.tile([128, ET, S], DT.bfloat16, tag="kT")
            v_sb = qkvp.tile([128, ST, HLOC, VW], DT.bfloat16, tag="v")
            nc.vector.memset(v_sb[:, :, :, D:D + 1], 1.0)
            ot_sb = qkvp.tile([128, NHP, S], DT.bfloat16, tag="ot")

            # preload the exp table set (~2.7us) during the head
            warm = norm2p.tile([1, 8], DT.float32, tag="warm")
            nc.vector.memset(warm, 0.0)
            nc.scalar.activation(out=warm, in_=warm, func=AF.Exp)

            # ---- projection units (generators: ~2 matmuls per step) ----
            def v_unit(st):
                pp = ps_proj.tile([128, E], DT.float32, tag="pp", name=f"vp{st}")
                for cb in range(CB):
                    nc.tensor.matmul(
                        pp,
                        xv_sb[:, cb, st * 128:(st + 1) * 128],
                        wv_sb[:, cb, :],
                        start=(cb == 0),
                        stop=(cb == CB - 1),
                    )
                nc.vector.tensor_copy(
                    v_sb[:, st, :, 0:D],
                    pp.rearrange("p (h d) -> p h d", h=HLOC),
                )

            def k_unit(et, sc):
                pp = ps_proj.tile([128, QCW], DT.float32, tag="pp",
                                  name=f"kp{et}_{sc}")
                for cb in range(CB):
                    nc.tensor.matmul(
                        pp,
                        wk_sb[:, cb, et * 128:(et + 1) * 128],
                        xk_sb[:, cb, sc * QCW:(sc + 1) * QCW],
                        start=(cb == 0),
                        stop=(cb == CB - 1),
                    )
                    if cb % 2 == 1 and cb < CB - 1:
                        yield
                nc.vector.tensor_copy(kT[:, et, sc * QCW:(sc + 1) * QCW], pp)

            def q_unit(qc, et):
                xq_sb = xq_tiles[qc]
                pp = ps_proj.tile([128, QCW], DT.float32, tag="pp",
                                  name=f"qp{qc}_{et}")
                for cb in range(CB):
                    nc.tensor.matmul(
                        pp,
                        wq_sb[:, cb, et * 128:(et + 1) * 128],
                        xq_sb[:, cb, :],
                        start=(cb == 0),
                        stop=(cb == CB - 1),
                    )
                    if cb % 2 == 1 and cb < CB - 1:
                        yield
                nc.vector.tensor_copy(qT[:, et, qc * QCW:(qc + 1) * QCW], pp)

            y_tiles = {}
            yp_tiles = {}

            def y_start(st, nch, n_hp):
                if nch == 0:
                    y_tiles[st] = youtp.tile(
                        [128, HID], DT.float32, tag="y", name=f"ysb{st}")
                yp = ps_proj.tile([128, QCW], DT.float32, tag="pp",
                                  name=f"yp{st}_{nch}")
                yp_tiles[(st, nch)] = yp
                for hp in range(n_hp):
                    nc.tensor.matmul(
                        yp,
                        ot_sb[:, hp, st * 128:(st + 1) * 128],
                        wo_sb[:, hp, nch * QCW:(nch + 1) * QCW],
                        start=(hp == 0),
                        stop=False,
                    )
                    if hp == 1:
                        yield

            def y_finish(st, nch, hp0):
                yp = yp_tiles.pop((st, nch))
                ysb = y_tiles[st]
                for hp in range(hp0, NHP):
                    nc.tensor.matmul(
                        yp,
                        ot_sb[:, hp, st * 128:(st + 1) * 128],
                        wo_sb[:, hp, nch * QCW:(nch + 1) * QCW],
                        start=False,
                        stop=(hp == NHP - 1),
                    )
                nc.vector.tensor_copy(ysb[:, nch * QCW:(nch + 1) * QCW], yp)
                if nch == 1:
                    nc.sync.dma_start(out=y[st * 128:(st + 1) * 128, :], in_=ysb)

            def y_unit(st, nch):
                yield from y_start(st, nch, 2)
                y_finish(st, nch, 2)

            def y_prefix(st, nch):
                yield from y_start(st, nch, 2)

            def dma_unit(fn):
                fn()
                return
                yield  # generator

            def skip(n):
                for _ in range(n):
                    yield

            junk_n = [0]

            def junk_unit(n):
                # dead score-shaped matmuls: keep the PE dense enough that
                # the HAM clock gate stays at full rate in thin rounds
                for i in range(n):
                    junk_n[0] += 1
                    jp = ps_s.tile([128, 2, QCW], DT.float32, tag="ps_s",
                                   name=f"junk{junk_n[0]}")
                    for par in range(2):
                        nc.tensor.matmul(
                            jp[:, par, :],
                            kT[par * D:par * D + D, 0, 0:128],
                            qT[par * D:par * D + D, 0, 0:QCW],
                            start=True,
                            stop=True,
                        )
                    yield

            # ---- serial head: V, K et0, Q qc0 et0..1 ----
            for st in range(ST):
                v_unit(st)
            for sc in range(SC):
                for _ in k_unit(0, sc):
                    pass
            for et in range(2):
                for _ in q_unit(0, et):
                    pass

            # ---- filler schedule (deque order; ~1 step per k-tile).
            # K(et,sc) before round hp=et reaches k-tile 4*sc (JIT);
            # Q(qc,et) before round 4qc+et; Y(qc,..) with hp3 >=4 k-tiles
            # after norm(qc,hp3) which is emitted at round 4qc+4, kt2.
            fillers = {
                0: [dma_unit(lambda: q_prep(1)), k_unit(1, 0), k_unit(1, 1), k_unit(1, 2)],
                1: [k_unit(1, 3), k_unit(2, 0), k_unit(2, 1), q_unit(0, 2)],
                2: [k_unit(2, 2), k_unit(2, 3), k_unit(3, 0), q_unit(0, 3)],
                3: [k_unit(3, 1), k_unit(3, 2), k_unit(3, 3), q_unit(1, 0)],
                4: [q_unit(1, 1), q_unit(1, 2), q_unit(1, 3),
                    dma_unit(lambda: q_prep(2)), junk_unit(2)],
                5: [y_unit(0, 0), y_unit(0, 1), y_unit(1, 0), q_unit(2, 0)],
                6: [y_unit(1, 1), y_unit(2, 0), y_unit(2, 1), q_unit(2, 1)],
                7: [y_unit(3, 0), y_unit(3, 1), q_unit(2, 2), q_unit(2, 3),
                    dma_unit(lambda: q_prep(3))],
                8: [q_unit(3, 0), junk_unit(6)],
                9: [y_unit(4, 0), y_unit(4, 1), y_unit(5, 0), q_unit(3, 1)],
                10: [y_unit(5, 1), y_unit(6, 0), y_unit(6, 1), q_unit(3, 2)],
                11: [y_unit(7, 0), y_unit(7, 1), q_unit(3, 3), junk_unit(3)],
                12: [junk_unit(10)],
                13: [y_unit(8, 0), y_unit(8, 1), y_unit(9, 0), y_unit(9, 1),
                     junk_unit(2)],
                14: [y_unit(10, 0), y_unit(10, 1), y_unit(11, 0), y_unit(11, 1),
                     junk_unit(2)],
                15: [junk_unit(7)],
            }
            pending = []

            # ---- attention: flat stream of 256 k-tile steps ----
            rounds = [(qc, hp) for qc in range(SC) for hp in range(NHP)]
            ov_tiles = {}
            ptq = []  # (r, kt, ptile)

            def attn_v(r, kt, ptile):
                qc, hp = rounds[r]
                if r not in ov_tiles:
                    ov_tiles[r] = [
                        ps_ov.tile([D + 1, QCW], DT.float32, tag=f"ov{par}",
                                   name=f"ov{r}_{par}")
                        for par in range(2)
                    ]
                ovs = ov_tiles[r]
                for par in range(2):
                    nc.tensor.matmul(
                        ovs[par],
                        v_sb[:, kt, 2 * hp + par, 0:D + 1],
                        ptile[:, par, :],
                        start=(kt == 0),
                        stop=(kt == ST - 1),
                    )

            norm_state = {}

            def norm_a(r):
                # copies only: releases the ov psum banks quickly and
                # keeps the DVE FIFO clear for filler drains
                ovs = ov_tiles.pop(r)
                ovsts = []
                for par in range(2):
                    ovst = norm2p.tile([D, QCW], DT.float32, tag=f"ovst{par}")
                    nc.vector.tensor_copy(ovst, ovs[par][0:D, :])
                    ovsts.append(ovst)
                sums = norm2p.tile([33, QCW], DT.float32, tag="sums")
                for par in range(2):
                    nc.vector.tensor_copy(
                        sums[32 * par:32 * par + 1, :], ovs[par][D:D + 1, :])
                norm_state[r] = (ovsts, sums)

            def norm_b(r):
                qc, hp = rounds[r]
                q0 = qc * QCW
                ovsts, sums = norm_state.pop(r)
                nc.vector.reciprocal(sums, sums)
                for par in range(2):
                    if par == 1:
                        nc.vector.tensor_copy(sums[0:1, :], sums[32:33, :])
                    bc = norm2p.tile([D, QCW], DT.float32, tag="bc")
                    nc.gpsimd.partition_broadcast(bc, sums[0:1, :])
                    nc.vector.tensor_mul(
                        ot_sb[par * D:par * D + D, hp, q0:q0 + QCW],
                        ovsts[par],
                        bc,
                    )

            for i in range(ST * len(rounds)):
                r, kt = divmod(i, ST)
                qc, hp = rounds[r]
                if kt == 0:
                    pending.extend(fillers.get(r, ()))
                sps = ps_s.tile([128, 2, QCW], DT.float32, tag="ps_s")
                for par in range(2):
                    prow = slice(par * D, par * D + D)
                    nc.tensor.matmul(
                        sps[:, par, :],
                        kT[prow, hp, kt * 128:(kt + 1) * 128],
                        qT[prow, hp, qc * QCW:(qc + 1) * QCW],
                        start=True,
                        stop=True,
                    )
                ptile = ptp.tile([128, 2, QCW], DT.bfloat16, tag="pt")
                nc.scalar.activation(
                    out=ptile, in_=sps, func=AF.Exp, scale=float(SCALE),
                )
                ptq.append((r, kt, ptile))
                if len(ptq) > LAG:
                    attn_v(*ptq.pop(0))
                if kt == LAG - 1 and r > 0:
                    norm_a(r - 1)
                if kt == LAG + 3 and r > 0:
                    norm_b(r - 1)
                # drip-feed ~2 filler matmuls into the PE slack
                if pending:
                    try:
                        next(pending[0])
                    except StopIteration:
                        pending.pop(0)

            while ptq:
                attn_v(*ptq.pop(0))
            for g in pending:
                for _ in g:
                    pass
            norm_a(15)
            # keep-warm bridge spanning the final norm chain: emitted
            # BEFORE the Y matmuls (whose weight-loads serialize on the
            # last norm multiply) so the PE queue isn't head-of-line
            # blocked and the HAM clock gate stays warm
            for _ in junk_unit(22):
                pass
            for nch in range(2):
                for _ in y_start(12, nch, 3):
                    pass
            norm_b(15)

            # ---- tail: finish Y of the last q-chunk ----
            y_finish(12, 0, 3)
            y_finish(12, 1, 3)
            for st in range(13, ST):
                for nch in range(2):
                    for _ in y_unit(st, nch):
                        pass

    nc.finalize()
    return nc


def _get_nc():
    with _lock:
        if "nc" not in _cache:
            _cache["nc"] = _build()
        return _cache["nc"]


def _in_maps(q, k, v, Wq, Wk, Wv, Wo):
    import ml_dtypes

    bf16 = ml_dtypes.bfloat16
    xT = {}
    for b in range(B):
        xT[b] = tuple(
            np.ascontiguousarray(t[b].astype(bf16).T) for t in (q, k, v)
        )
    w_bf = [
        (np.ascontiguousarray(W[:, hf * E:(hf + 1) * E].astype(bf16)) if W is not Wo
         else np.ascontiguousarray(W[hf * E:(hf + 1) * E, :].astype(bf16)))
        for hf in range(2) for W in (Wq, Wk, Wv, Wo)
    ]
    maps = []
    for c in range(8):
        b, hf = c // 2, c % 2
        qt, kt, vt = xT[b]
        wqc, wkc, wvc, woc = w_bf[hf * 4:(hf + 1) * 4]
        maps.append({
            "xqT": qt,
            "xkT": kt,
            "xvT": vt,
            "wq": wqc,
            "wk": wkc,
            "wv": wvc,
            "wo": woc,
        })
    return maps


def run(q, k, v, Wq, Wk, Wv, Wo, **spmd_kwargs):
    nc = _get_nc()
    res = run_bass_kernel_spmd(
        nc, _in_maps(q, k, v, Wq, Wk, Wv, Wo), core_ids=list(range(8)),
        **spmd_kwargs,
    )
    out = np.empty((B, S, HID), dtype=np.float32)
    for b in range(B):
        out[b] = res.results[2 * b]["y"] + res.results[2 * b + 1]["y"]
    return out, res


def kernel(q, k, v, Wq, Wk, Wv, Wo):
    out, _ = run(q, k, v, Wq, Wk, Wv, Wo)
    return out
